# revision 7
# baseline (speedup 1.0000x reference)
"""Trainium2 Bass kernel for the counting-criterion loss.

Computes, for output/density_map of shape [32, 1, 512, 512] and bboxes [32, 3, 4]:
  dmap_loss  = sum((output - density_map)^2) / num_objects
  count_loss = mean_b((sum(output_b) - sum(density_map_b))^2)
  min_count  = sum_boxes(relu(1 - box_sum))   with box sums over [y1:y2, x1:x2)

Strategy: data-parallel over the batch - core i handles images [4i, 4i+4).
Inputs are downcast to bf16 on the host (loss tolerance is 2e-2; bf16
quantization contributes ~1e-3 relative), halving the HBM stream, which is
the roofline for this kernel.

Per core (4 images), per piece of the DMA stream:
  - diff + per-partition sum(diff): one DVE tensor_tensor_reduce (a couple of
    mid-stream halves run on gpsimd scalar_tensor_tensor instead to keep DVE
    clear for the tail)
  - sum(diff^2): PE Gram-matrix trick - accumulate diff_blk^T @ diff_blk over
    all [128,128] blocks of an image into one [128,128] PSUM tile; its
    diagonal is sum(diff^2) per x-residue, extracted with a single [128,128]
    eye-mask multiply (eye built on-device via affine_select). The final two
    tail pieces square on DVE directly so the last dependency chain is short.
  - box row-sums on PE: o_blk (lhsT) @ rowmask -> ps[x, (cx,j)], accumulated
    over y-chunks; ps is copied raw to the output and the column mask +
    final reductions run on the host.
The DMA stream shrinks geometrically at the end (image 3 leaves in 5 pieces)
and one quarter-pair is issued through the Pool SWDGE queue, which keeps the
8-semaphore HWDGE recycling window stall-free.
"""

import numpy as np
from contextlib import ExitStack

import concourse.bass as bass
import concourse.mybir as mybir
import concourse.tile as tile
from concourse import bacc
from concourse.bass_utils import run_bass_kernel_spmd

N_CORES = 8
B, H, W = 32, 512, 512
NIMG = B // N_CORES  # images per core
P = 128              # SBUF partitions
NCH = H // P         # row chunks per image (and col chunks: W//P)
NB = 3               # boxes per image
F32 = mybir.dt.float32
BF16 = mybir.dt.bfloat16

# diff pieces: (img, c0, c1, x0, x1) in emission order of their accum columns
DIFF_PIECES = [
    (3, 0, 1, 0, 512),    # q0 (pool-DMA'd, lands early)
    (0, 0, 4, 0, 512),    # img0 full
    (1, 0, 2, 0, 512),    # img1 h0
    (1, 2, 4, 0, 512),    # img1 h1 (diff on Pool)
    (2, 0, 2, 0, 512),    # img2 h0
    (2, 2, 4, 0, 512),    # img2 h1 (diff on Pool)
    (3, 1, 2, 0, 512),    # q1
    (3, 2, 3, 0, 512),    # q2
    (3, 3, 4, 0, 384),    # e3a (diff on Pool, square on ACT)
    (3, 3, 4, 384, 512),  # e3b (diff + square on DVE)
]
NDIFF = len(DIFF_PIECES)
# square columns: 4 per-image PSUM diagonals + e3a + e3b direct squares
NSQ = NIMG + 2
NBOXCOL = NIMG * NCH * NB
NACC = NDIFF + NSQ + NBOXCOL

_PROG = None


def _build_program():
    nc = bacc.Bacc(
        "TRN2",
        target_bir_lowering=False,
        debug=False,
        num_devices=N_CORES,
    )
    o_d = nc.dram_tensor("o", [NIMG, H, W], BF16, kind="ExternalInput").ap()
    d_d = nc.dram_tensor("d", [NIMG, H, W], BF16, kind="ExternalInput").ap()
    # row mask, interleaved on host: rm[p, (img, cy, j)]
    rm_d = nc.dram_tensor("rm", [P, NIMG * NCH * NB], BF16, kind="ExternalInput").ap()
    acc_d = nc.dram_tensor("acc", [P, NACC], F32, kind="ExternalOutput").ap()

    o_r = o_d.rearrange("n (c p) x -> n p c x", p=P)
    d_r = d_d.rearrange("n (c p) x -> n p c x", p=P)

    with tile.TileContext(nc) as tc, ExitStack() as ctx:
        io_pool = ctx.enter_context(tc.tile_pool(name="io", bufs=4))
        work_pool = ctx.enter_context(tc.tile_pool(name="work", bufs=4))
        mask_pool = ctx.enter_context(tc.tile_pool(name="mask", bufs=1))
        psum_pool = ctx.enter_context(tc.tile_pool(name="psum", bufs=4, space="PSUM"))
        acc_pool = ctx.enter_context(tc.tile_pool(name="acc", bufs=1))

        acc = acc_pool.tile([P, NACC], F32)
        rm_t = mask_pool.tile([P, NIMG * NCH * NB], BF16)
        ones_t = mask_pool.tile([P, P], F32)
        eye_t = mask_pool.tile([P, P], F32)

        o_t = {i: io_pool.tile([P, NCH, W], BF16, tag="o", name=f"o{i}") for i in range(NIMG)}
        d_t = {i: io_pool.tile([P, NCH, W], BF16, tag="d", name=f"d{i}") for i in range(NIMG)}
        diff_t = {i: work_pool.tile([P, NCH, W], BF16, tag="diff", name=f"diff{i}") for i in range(NIMG)}
        sqs_t = {i: work_pool.tile([P, 512], BF16, tag="sqs", name=f"sqs{i}") for i in range(2)}
        escr_t = {i: work_pool.tile([P, P], F32, tag="escr", name=f"escr{i}") for i in range(NIMG)}
        boxps = {i: psum_pool.tile([P, NCH * NB], F32, tag="boxps", name=f"boxps{i}") for i in range(NIMG)}
        sqps = {i: psum_pool.tile([P, P], F32, tag="sqps", name=f"sqps{i}") for i in range(NIMG)}

        pidx = {pc: i for i, pc in enumerate(DIFF_PIECES)}

        def dma_pair(img, c0, c1, x0, x1, eng=nc.sync):
            eng.dma_start(o_t[img][:, c0:c1, x0:x1], o_r[img, :, c0:c1, x0:x1])
            eng.dma_start(d_t[img][:, c0:c1, x0:x1], d_r[img, :, c0:c1, x0:x1])

        def diff_op(pc, engine):
            img, c0, c1, x0, x1 = pc
            col = pidx[pc]
            if engine == "V":
                nc.vector.tensor_tensor_reduce(
                    out=diff_t[img][:, c0:c1, x0:x1],
                    in0=o_t[img][:, c0:c1, x0:x1],
                    in1=d_t[img][:, c0:c1, x0:x1],
                    scale=1.0,
                    scalar=0.0,
                    op0=mybir.AluOpType.subtract,
                    op1=mybir.AluOpType.add,
                    accum_out=acc[:, col : col + 1],
                )
            else:
                nc.gpsimd.scalar_tensor_tensor(
                    out=diff_t[img][:, c0:c1, x0:x1],
                    in0=o_t[img][:, c0:c1, x0:x1],
                    scalar=0.0,
                    in1=d_t[img][:, c0:c1, x0:x1],
                    op0=mybir.AluOpType.bypass,
                    op1=mybir.AluOpType.subtract,
                    accum_out=acc[:, col : col + 1],
                )

        def box_mms(img, c0, c1, x0, x1):
            for cy in range(c0, c1):
                for cx in range(x0 // P, x1 // P):
                    nc.tensor.matmul(
                        boxps[img][:, cx * NB : (cx + 1) * NB],
                        lhsT=o_t[img][:, cy, cx * P : (cx + 1) * P],
                        rhs=rm_t[:, (img * NCH + cy) * NB : (img * NCH + cy + 1) * NB],
                        start=(cy == 0),
                        stop=(cy == NCH - 1),
                    )

        def sq_mms(img, c0, c1, x0, x1, start=False, stop=False):
            blocks = [(cy, cx) for cy in range(c0, c1) for cx in range(x0 // P, x1 // P)]
            for k, (cy, cx) in enumerate(blocks):
                blk = diff_t[img][:, cy, cx * P : (cx + 1) * P]
                nc.tensor.matmul(
                    sqps[img][:],
                    lhsT=blk,
                    rhs=blk,
                    start=start and k == 0,
                    stop=stop and k == len(blocks) - 1,
                )

        def eye_extract(img, engine):
            # acc col = sum_r sqps[p, r] * eye[p, r] = sqps[p, p]
            col = NDIFF + img
            kw = dict(
                out=escr_t[img][:],
                in0=sqps[img][:],
                scalar=0.0,
                in1=eye_t[:],
                op0=mybir.AluOpType.bypass,
                op1=mybir.AluOpType.mult,
                accum_out=acc[:, col : col + 1],
            )
            if engine == "V":
                nc.vector.scalar_tensor_tensor(**kw)
            else:
                nc.gpsimd.scalar_tensor_tensor(**kw)

        def ps_copy(img):
            col0 = NDIFF + NSQ + img * NCH * NB
            nc.gpsimd.tensor_copy(acc[:, col0 : col0 + NCH * NB], boxps[img][:])

        # eye mask: out[p, x] = (p - x == 0) ? 1.0 : 0.0
        nc.vector.memset(ones_t[:], 1.0)
        nc.gpsimd.affine_select(
            out=eye_t[:],
            in_=ones_t[:],
            pattern=[[-1, P]],
            compare_op=mybir.AluOpType.is_equal,
            fill=0.0,
            base=0,
            channel_multiplier=1,
        )

        # ---- DMA stream ----
        # HW (SP): o0 rm | d0 o1 d1 o2h0 d2h0 o2h1 d2h1 q1 q2 e3a e3b pairs
        # Pool SWDGE: q0 pair (desc-gen runs early, transfers land after o0)
        nc.sync.dma_start(o_t[0][:], o_r[0])
        nc.scalar.dma_start(rm_t[:], rm_d)
        dma_pair(3, 0, 1, 0, 512, eng=nc.gpsimd)  # q0 via SWDGE
        nc.sync.dma_start(d_t[0][:], d_r[0])
        dma_pair(1, 0, 4, 0, 512)
        dma_pair(2, 0, 2, 0, 512)
        dma_pair(2, 2, 4, 0, 512)
        dma_pair(3, 1, 2, 0, 512)  # q1
        dma_pair(3, 2, 3, 0, 512)  # q2
        dma_pair(3, 3, 4, 0, 384)  # e3a
        dma_pair(3, 3, 4, 384, 512)  # e3b

        # ---- compute, in per-engine readiness order ----
        # DVE queue
        diff_op((3, 0, 1, 0, 512), "V")       # q0
        diff_op((0, 0, 4, 0, 512), "V")       # img0
        diff_op((1, 0, 2, 0, 512), "V")       # img1 h0
        diff_op((2, 0, 2, 0, 512), "V")       # img2 h0
        eye_extract(0, "V")
        eye_extract(1, "V")
        diff_op((3, 1, 2, 0, 512), "V")       # q1
        diff_op((3, 2, 3, 0, 512), "V")       # q2
        diff_op((3, 3, 4, 384, 512), "V")     # e3b
        nc.vector.scalar_tensor_tensor(       # e3b square on DVE
            out=sqs_t[1][:, 384:512],
            in0=diff_t[3][:, 3, 384:512],
            scalar=0.0,
            in1=diff_t[3][:, 3, 384:512],
            op0=mybir.AluOpType.bypass,
            op1=mybir.AluOpType.mult,
            accum_out=acc[:, NDIFF + NIMG + 1 : NDIFF + NIMG + 2],
        )

        # Pool queue (after the eye select + q0 DGE above)
        ps_copy(0)
        diff_op((1, 2, 4, 0, 512), "P")       # img1 h1
        ps_copy(1)
        diff_op((2, 2, 4, 0, 512), "P")       # img2 h1
        ps_copy(2)
        diff_op((3, 3, 4, 0, 384), "P")       # e3a
        ps_copy(3)
        eye_extract(2, "P")
        eye_extract(3, "P")

        # ACT queue
        nc.scalar.activation(                  # e3a square on ACT
            sqs_t[0][:, 0:384],
            diff_t[3][:, 3, 0:384],
            mybir.ActivationFunctionType.Square,
            accum_out=acc[:, NDIFF + NIMG : NDIFF + NIMG + 1],
        )

        # PE queue, readiness order
        box_mms(0, 0, 4, 0, 512)
        box_mms(3, 0, 1, 0, 512)              # q0 blocks (cy0)
        sq_mms(3, 0, 1, 0, 512, start=True)   # img3 sq: q0
        box_mms(1, 0, 4, 0, 512)
        sq_mms(0, 0, 4, 0, 512, start=True, stop=True)
        box_mms(2, 0, 2, 0, 512)
        sq_mms(1, 0, 2, 0, 512, start=True)
        sq_mms(1, 2, 4, 0, 512, stop=True)
        box_mms(2, 2, 4, 0, 512)
        sq_mms(2, 0, 2, 0, 512, start=True)
        box_mms(3, 1, 2, 0, 512)              # q1 blocks (cy1)
        sq_mms(3, 1, 2, 0, 512)
        box_mms(3, 2, 3, 0, 512)              # q2 blocks (cy2)
        sq_mms(2, 2, 4, 0, 512, stop=True)
        box_mms(3, 3, 4, 0, 384)              # cy3, cx0-2
        box_mms(3, 3, 4, 384, 512)            # cy3, cx3
        sq_mms(3, 2, 3, 0, 512, stop=True)    # img3 sq: q2 (stop)

        nc.sync.dma_start(acc_d, acc[:])

    nc.compile()
    return nc


def _get_program():
    global _PROG
    if _PROG is None:
        _PROG = _build_program()
    return _PROG


def _prep_inputs(output, density_map, bboxes):
    import ml_dtypes

    o = np.asarray(output, dtype=np.float32).reshape(B, H, W).astype(ml_dtypes.bfloat16)
    dm = (
        np.asarray(density_map, dtype=np.float32)
        .reshape(B, H, W)
        .astype(ml_dtypes.bfloat16)
    )
    o = np.ascontiguousarray(o)
    dm = np.ascontiguousarray(dm)
    bb = np.clip(np.asarray(bboxes).astype(np.int64), 0, W).astype(np.int32)
    x1, y1, x2, y2 = bb[..., 0], bb[..., 1], bb[..., 2], bb[..., 3]
    x2 = np.maximum(x2, x1)
    y2 = np.maximum(y2, y1)

    ar = np.arange(H, dtype=np.int32)
    # rm[b, y, j] = 1 if y1 <= y < y2, stored per core as [p, (img, cy, j)]
    rm = (
        (ar[None, :, None] >= y1[:, None, :]) & (ar[None, :, None] < y2[:, None, :])
    ).astype(np.float32)
    rm = rm.reshape(B, NCH, P, NB).transpose(0, 2, 1, 3)  # [B, p, cy, j]
    rm = rm.reshape(N_CORES, NIMG, P, NCH * NB).transpose(0, 2, 1, 3)
    rm = np.ascontiguousarray(
        rm.reshape(N_CORES, P, NIMG * NCH * NB).astype(ml_dtypes.bfloat16)
    )
    # column mask stays on the host: cm[b, x, j]
    cm = (
        (ar[None, :, None] >= x1[:, None, :]) & (ar[None, :, None] < x2[:, None, :])
    ).astype(np.float64)
    return o, dm, rm, cm


def kernel(output, density_map, bboxes, num_objects):
    o, dm, rm, cm = _prep_inputs(output, density_map, bboxes)

    nc = _get_program()
    in_maps = [
        {
            "o": o[i * NIMG : (i + 1) * NIMG],
            "d": dm[i * NIMG : (i + 1) * NIMG],
            "rm": rm[i],
        }
        for i in range(N_CORES)
    ]
    res = run_bass_kernel_spmd(nc, in_maps, core_ids=list(range(N_CORES)))

    per_img_diff = np.zeros(B, dtype=np.float64)
    sq_total = 0.0
    box_sums = np.zeros((B, NB), dtype=np.float64)
    for core, r in enumerate(res.results):
        a = r["acc"].astype(np.float64)
        for i, pc in enumerate(DIFF_PIECES):
            per_img_diff[core * NIMG + pc[0]] += a[:, i].sum()
        sq_total += a[:, NDIFF : NDIFF + NSQ].sum()
        ps = a[:, NDIFF + NSQ :].reshape(P, NIMG, NCH, NB)
        for li in range(NIMG):
            b = core * NIMG + li
            cmb = cm[b].reshape(NCH, P, NB)  # [cx, p, j]
            box_sums[b] += np.einsum("pcj,cpj->j", ps[:, li], cmb)

    dmap_loss = sq_total / float(num_objects)
    count_loss = float(np.mean(per_img_diff**2))
    min_count = float(np.maximum(0.0, 1.0 - box_sums).sum())
    return np.array([dmap_loss, count_loss, min_count], dtype=np.float32)


# revision 8
# speedup vs baseline: 1.0151x; 1.0151x over previous
"""Trainium2 Bass kernel for the counting-criterion loss.

Computes, for output/density_map of shape [32, 1, 512, 512] and bboxes [32, 3, 4]:
  dmap_loss  = sum((output - density_map)^2) / num_objects
  count_loss = mean_b((sum(output_b) - sum(density_map_b))^2)
  min_count  = sum_boxes(relu(1 - box_sum))   with box sums over [y1:y2, x1:x2)

Strategy: data-parallel over the batch - core i handles images [4i, 4i+4).
Inputs are downcast to bf16 on the host (loss tolerance is 2e-2; bf16
quantization contributes ~1e-3 relative), halving the HBM stream, which is
the roofline for this kernel.

Per core (4 images), per piece of the DMA stream:
  - diff + per-partition sum(diff): one DVE tensor_tensor_reduce (a couple of
    mid-stream halves run on gpsimd scalar_tensor_tensor instead to keep DVE
    clear for the tail)
  - sum(diff^2): PE Gram-matrix trick - accumulate diff_blk^T @ diff_blk over
    all [128,128] blocks of an image into one [128,128] PSUM tile; its
    diagonal is sum(diff^2) per x-residue, extracted with a single [128,128]
    eye-mask multiply (eye built on-device via affine_select). The final two
    tail pieces square on DVE directly so the last dependency chain is short.
  - box row-sums on PE: o_blk (lhsT) @ rowmask -> ps[x, (cx,j)], accumulated
    over y-chunks; ps is copied raw to the output and the column mask +
    final reductions run on the host.
The DMA stream shrinks geometrically at the end (image 3 leaves in 5 pieces)
and one quarter-pair is issued through the Pool SWDGE queue, which keeps the
8-semaphore HWDGE recycling window stall-free.
"""

import numpy as np
from contextlib import ExitStack

import concourse.bass as bass
import concourse.mybir as mybir
import concourse.tile as tile
from concourse import bacc
from concourse.bass_utils import run_bass_kernel_spmd

N_CORES = 8
B, H, W = 32, 512, 512
NIMG = B // N_CORES  # images per core
P = 128              # SBUF partitions
NCH = H // P         # row chunks per image (and col chunks: W//P)
NB = 3               # boxes per image
F32 = mybir.dt.float32
BF16 = mybir.dt.bfloat16

# diff pieces: (img, c0, c1, x0, x1) in emission order of their accum columns
DIFF_PIECES = [
    (3, 0, 1, 0, 512),    # q0 (pool-DMA'd, lands early)
    (0, 0, 4, 0, 512),    # img0 full
    (1, 0, 2, 0, 512),    # img1 h0
    (1, 2, 4, 0, 512),    # img1 h1 (diff on Pool)
    (2, 0, 2, 0, 512),    # img2 h0
    (2, 2, 4, 0, 512),    # img2 h1 (diff on Pool)
    (3, 1, 2, 0, 512),    # q1
    (3, 2, 3, 0, 512),    # q2
    (3, 3, 4, 0, 256),    # e3a (diff on Pool, square on ACT)
    (3, 3, 4, 256, 512),  # e3b (diff + square on DVE)
]
NDIFF = len(DIFF_PIECES)
# square columns: 4 per-image PSUM diagonals + e3a + e3b direct squares
NSQ = NIMG + 2
NBOXCOL = NIMG * NCH * NB
NACC = NDIFF + NSQ + NBOXCOL

_PROG = None


def _build_program():
    nc = bacc.Bacc(
        "TRN2",
        target_bir_lowering=False,
        debug=False,
        num_devices=N_CORES,
    )
    o_d = nc.dram_tensor("o", [NIMG, H, W], BF16, kind="ExternalInput").ap()
    d_d = nc.dram_tensor("d", [NIMG, H, W], BF16, kind="ExternalInput").ap()
    # row mask, interleaved on host: rm[p, (img, cy, j)]
    rm_d = nc.dram_tensor("rm", [P, NIMG * NCH * NB], BF16, kind="ExternalInput").ap()
    acc_d = nc.dram_tensor("acc", [P, NACC], F32, kind="ExternalOutput").ap()

    o_r = o_d.rearrange("n (c p) x -> n p c x", p=P)
    d_r = d_d.rearrange("n (c p) x -> n p c x", p=P)

    with tile.TileContext(nc) as tc, ExitStack() as ctx:
        io_pool = ctx.enter_context(tc.tile_pool(name="io", bufs=4))
        work_pool = ctx.enter_context(tc.tile_pool(name="work", bufs=4))
        mask_pool = ctx.enter_context(tc.tile_pool(name="mask", bufs=1))
        psum_pool = ctx.enter_context(tc.tile_pool(name="psum", bufs=4, space="PSUM"))
        acc_pool = ctx.enter_context(tc.tile_pool(name="acc", bufs=1))

        acc = acc_pool.tile([P, NACC], F32)
        rm_t = mask_pool.tile([P, NIMG * NCH * NB], BF16)
        ones_t = mask_pool.tile([P, P], F32)
        eye_t = mask_pool.tile([P, P], F32)

        o_t = {i: io_pool.tile([P, NCH, W], BF16, tag="o", name=f"o{i}") for i in range(NIMG)}
        d_t = {i: io_pool.tile([P, NCH, W], BF16, tag="d", name=f"d{i}") for i in range(NIMG)}
        diff_t = {i: work_pool.tile([P, NCH, W], BF16, tag="diff", name=f"diff{i}") for i in range(NIMG)}
        sqs_t = {i: work_pool.tile([P, 512], BF16, tag="sqs", name=f"sqs{i}") for i in range(2)}
        escr_t = {i: work_pool.tile([P, P], F32, tag="escr", name=f"escr{i}") for i in range(NIMG)}
        boxps = {i: psum_pool.tile([P, NCH * NB], F32, tag="boxps", name=f"boxps{i}") for i in range(NIMG)}
        sqps = {i: psum_pool.tile([P, P], F32, tag="sqps", name=f"sqps{i}") for i in range(NIMG)}

        pidx = {pc: i for i, pc in enumerate(DIFF_PIECES)}

        def dma_pair(img, c0, c1, x0, x1, eng=nc.sync):
            eng.dma_start(o_t[img][:, c0:c1, x0:x1], o_r[img, :, c0:c1, x0:x1])
            eng.dma_start(d_t[img][:, c0:c1, x0:x1], d_r[img, :, c0:c1, x0:x1])

        def diff_op(pc, engine):
            img, c0, c1, x0, x1 = pc
            col = pidx[pc]
            if engine == "V":
                nc.vector.tensor_tensor_reduce(
                    out=diff_t[img][:, c0:c1, x0:x1],
                    in0=o_t[img][:, c0:c1, x0:x1],
                    in1=d_t[img][:, c0:c1, x0:x1],
                    scale=1.0,
                    scalar=0.0,
                    op0=mybir.AluOpType.subtract,
                    op1=mybir.AluOpType.add,
                    accum_out=acc[:, col : col + 1],
                )
            else:
                nc.gpsimd.scalar_tensor_tensor(
                    out=diff_t[img][:, c0:c1, x0:x1],
                    in0=o_t[img][:, c0:c1, x0:x1],
                    scalar=0.0,
                    in1=d_t[img][:, c0:c1, x0:x1],
                    op0=mybir.AluOpType.bypass,
                    op1=mybir.AluOpType.subtract,
                    accum_out=acc[:, col : col + 1],
                )

        def box_mms(img, c0, c1, x0, x1):
            for cy in range(c0, c1):
                for cx in range(x0 // P, x1 // P):
                    nc.tensor.matmul(
                        boxps[img][:, cx * NB : (cx + 1) * NB],
                        lhsT=o_t[img][:, cy, cx * P : (cx + 1) * P],
                        rhs=rm_t[:, (img * NCH + cy) * NB : (img * NCH + cy + 1) * NB],
                        start=(cy == 0),
                        stop=(cy == NCH - 1),
                    )

        def sq_mms(img, c0, c1, x0, x1, start=False, stop=False):
            blocks = [(cy, cx) for cy in range(c0, c1) for cx in range(x0 // P, x1 // P)]
            for k, (cy, cx) in enumerate(blocks):
                blk = diff_t[img][:, cy, cx * P : (cx + 1) * P]
                nc.tensor.matmul(
                    sqps[img][:],
                    lhsT=blk,
                    rhs=blk,
                    start=start and k == 0,
                    stop=stop and k == len(blocks) - 1,
                )

        def eye_extract(img, engine):
            # acc col = sum_r sqps[p, r] * eye[p, r] = sqps[p, p]
            col = NDIFF + img
            kw = dict(
                out=escr_t[img][:],
                in0=sqps[img][:],
                scalar=0.0,
                in1=eye_t[:],
                op0=mybir.AluOpType.bypass,
                op1=mybir.AluOpType.mult,
                accum_out=acc[:, col : col + 1],
            )
            if engine == "V":
                nc.vector.scalar_tensor_tensor(**kw)
            else:
                nc.gpsimd.scalar_tensor_tensor(**kw)

        def ps_copy(img):
            col0 = NDIFF + NSQ + img * NCH * NB
            nc.gpsimd.tensor_copy(acc[:, col0 : col0 + NCH * NB], boxps[img][:])

        # ---- DMA stream ----
        # HW (SP): o0 rm | d0 o1 d1 o2h0 d2h0 o2h1 d2h1 q1 q2 e3a e3b pairs
        # Pool SWDGE: q0 pair (desc-gen runs early, transfers land after o0)
        nc.sync.dma_start(o_t[0][:], o_r[0])
        nc.scalar.dma_start(rm_t[:], rm_d)
        dma_pair(3, 0, 1, 0, 512, eng=nc.gpsimd)  # q0 via SWDGE
        nc.sync.dma_start(d_t[0][:], d_r[0])
        dma_pair(1, 0, 4, 0, 512)
        dma_pair(2, 0, 2, 0, 512)
        dma_pair(2, 2, 4, 0, 512)
        dma_pair(3, 1, 2, 0, 512)  # q1
        dma_pair(3, 2, 3, 0, 512)  # q2
        dma_pair(3, 3, 4, 0, 256)  # e3a
        dma_pair(3, 3, 4, 256, 512)  # e3b

        # eye mask: out[p, x] = (p - x == 0) ? 1.0 : 0.0
        nc.vector.memset(ones_t[:], 1.0)
        nc.gpsimd.affine_select(
            out=eye_t[:],
            in_=ones_t[:],
            pattern=[[-1, P]],
            compare_op=mybir.AluOpType.is_equal,
            fill=0.0,
            base=0,
            channel_multiplier=1,
        )


        # ---- compute, in per-engine readiness order ----
        # DVE queue
        diff_op((3, 0, 1, 0, 512), "V")       # q0
        diff_op((0, 0, 4, 0, 512), "V")       # img0
        diff_op((1, 0, 2, 0, 512), "V")       # img1 h0
        diff_op((2, 0, 2, 0, 512), "V")       # img2 h0
        eye_extract(0, "V")
        eye_extract(1, "V")
        diff_op((3, 1, 2, 0, 512), "V")       # q1
        diff_op((3, 2, 3, 0, 512), "V")       # q2
        diff_op((3, 3, 4, 256, 512), "V")     # e3b
        nc.vector.scalar_tensor_tensor(       # e3b square on DVE
            out=sqs_t[1][:, 256:512],
            in0=diff_t[3][:, 3, 256:512],
            scalar=0.0,
            in1=diff_t[3][:, 3, 256:512],
            op0=mybir.AluOpType.bypass,
            op1=mybir.AluOpType.mult,
            accum_out=acc[:, NDIFF + NIMG + 1 : NDIFF + NIMG + 2],
        )

        # Pool queue (after the eye select + q0 DGE above)
        ps_copy(0)
        diff_op((1, 2, 4, 0, 512), "P")       # img1 h1
        ps_copy(1)
        diff_op((2, 2, 4, 0, 512), "P")       # img2 h1
        ps_copy(2)
        diff_op((3, 3, 4, 0, 256), "P")       # e3a
        ps_copy(3)
        eye_extract(2, "P")
        eye_extract(3, "P")

        # ACT queue
        nc.scalar.activation(                  # e3a square on ACT
            sqs_t[0][:, 0:256],
            diff_t[3][:, 3, 0:256],
            mybir.ActivationFunctionType.Square,
            accum_out=acc[:, NDIFF + NIMG : NDIFF + NIMG + 1],
        )

        # PE queue, readiness order
        box_mms(0, 0, 4, 0, 512)
        box_mms(3, 0, 1, 0, 512)              # q0 blocks (cy0)
        sq_mms(3, 0, 1, 0, 512, start=True)   # img3 sq: q0
        box_mms(1, 0, 4, 0, 512)
        sq_mms(0, 0, 4, 0, 512, start=True, stop=True)
        box_mms(2, 0, 2, 0, 512)
        sq_mms(1, 0, 2, 0, 512, start=True)
        sq_mms(1, 2, 4, 0, 512, stop=True)
        box_mms(2, 2, 4, 0, 512)
        sq_mms(2, 0, 2, 0, 512, start=True)
        box_mms(3, 1, 2, 0, 512)              # q1 blocks (cy1)
        sq_mms(3, 1, 2, 0, 512)
        box_mms(3, 2, 3, 0, 512)              # q2 blocks (cy2)
        sq_mms(2, 2, 4, 0, 512, stop=True)
        box_mms(3, 3, 4, 0, 256)              # cy3, cx0-1
        box_mms(3, 3, 4, 256, 512)            # cy3, cx2-3
        sq_mms(3, 2, 3, 0, 512, stop=True)    # img3 sq: q2 (stop)

        nc.sync.dma_start(acc_d, acc[:])

    nc.compile()
    return nc


def _get_program():
    global _PROG
    if _PROG is None:
        _PROG = _build_program()
    return _PROG


def _prep_inputs(output, density_map, bboxes):
    import ml_dtypes

    o = np.asarray(output, dtype=np.float32).reshape(B, H, W).astype(ml_dtypes.bfloat16)
    dm = (
        np.asarray(density_map, dtype=np.float32)
        .reshape(B, H, W)
        .astype(ml_dtypes.bfloat16)
    )
    o = np.ascontiguousarray(o)
    dm = np.ascontiguousarray(dm)
    bb = np.clip(np.asarray(bboxes).astype(np.int64), 0, W).astype(np.int32)
    x1, y1, x2, y2 = bb[..., 0], bb[..., 1], bb[..., 2], bb[..., 3]
    x2 = np.maximum(x2, x1)
    y2 = np.maximum(y2, y1)

    ar = np.arange(H, dtype=np.int32)
    # rm[b, y, j] = 1 if y1 <= y < y2, stored per core as [p, (img, cy, j)]
    rm = (
        (ar[None, :, None] >= y1[:, None, :]) & (ar[None, :, None] < y2[:, None, :])
    ).astype(np.float32)
    rm = rm.reshape(B, NCH, P, NB).transpose(0, 2, 1, 3)  # [B, p, cy, j]
    rm = rm.reshape(N_CORES, NIMG, P, NCH * NB).transpose(0, 2, 1, 3)
    rm = np.ascontiguousarray(
        rm.reshape(N_CORES, P, NIMG * NCH * NB).astype(ml_dtypes.bfloat16)
    )
    # column mask stays on the host: cm[b, x, j]
    cm = (
        (ar[None, :, None] >= x1[:, None, :]) & (ar[None, :, None] < x2[:, None, :])
    ).astype(np.float64)
    return o, dm, rm, cm


def kernel(output, density_map, bboxes, num_objects):
    o, dm, rm, cm = _prep_inputs(output, density_map, bboxes)

    nc = _get_program()
    in_maps = [
        {
            "o": o[i * NIMG : (i + 1) * NIMG],
            "d": dm[i * NIMG : (i + 1) * NIMG],
            "rm": rm[i],
        }
        for i in range(N_CORES)
    ]
    res = run_bass_kernel_spmd(nc, in_maps, core_ids=list(range(N_CORES)))

    per_img_diff = np.zeros(B, dtype=np.float64)
    sq_total = 0.0
    box_sums = np.zeros((B, NB), dtype=np.float64)
    for core, r in enumerate(res.results):
        a = r["acc"].astype(np.float64)
        for i, pc in enumerate(DIFF_PIECES):
            per_img_diff[core * NIMG + pc[0]] += a[:, i].sum()
        sq_total += a[:, NDIFF : NDIFF + NSQ].sum()
        ps = a[:, NDIFF + NSQ :].reshape(P, NIMG, NCH, NB)
        for li in range(NIMG):
            b = core * NIMG + li
            cmb = cm[b].reshape(NCH, P, NB)  # [cx, p, j]
            box_sums[b] += np.einsum("pcj,cpj->j", ps[:, li], cmb)

    dmap_loss = sq_total / float(num_objects)
    count_loss = float(np.mean(per_img_diff**2))
    min_count = float(np.maximum(0.0, 1.0 - box_sums).sum())
    return np.array([dmap_loss, count_loss, min_count], dtype=np.float32)
